# revision 1
# baseline (speedup 1.0000x reference)
"""Multi-head attention (B=2, S=2048, D=768, H=12) on 8 NeuronCores.

Sharding: data-parallel over batch (2) x tensor-parallel over heads (4 groups
of 3 heads) = 8 cores. Each core computes its 3 heads' Q/K/V projections,
attention, and a partial output projection; the host sums the 4 per-batch
partials and adds the output bias.

Per-core kernel layout (all matmuls in float32r: 1 cycle/row at N>=256):
  xT   [768, 2048]  input transposed (d on partitions, 6 chunks of 128)
  QT,KT[192, 2048]  transposed projections (head-major rows, bias via
                    rank-1 ones matmul)
  V    [2048, 3x65] natural-layout V with a ones column appended per head:
                    the ctx matmul lhsT [sk, 65] then yields softmax
                    denominators in PSUM row 64 for free.
  scoresT [sk 128, sq] per (head, sk-chunk) in PSUM -> Exp on ScalarE
                    (scale=1/sqrt(dk) folded into the activation) -> SBUF
  ctxT accumulates over sk in PSUM [65, 512] per sq-chunk; normalized on
                    eviction via reciprocal_approx_fast + partition_broadcast
  outT [768, 2048] partial output projection, host-summed across head groups
"""

import sys

sys.path.insert(0, "/opt/trn_rl_repo")

import numpy as np

B, S, D = 2, 2048, 768
H, DK = 12, 64
P = 128
HG = 3              # heads per core
E = HG * DK         # 192: per-core projection width
KD = D // P         # 6 contraction chunks
SQC = S // 512      # 4 sq chunks of 512
SKC = S // P        # 16 sk chunks of 128
SCALE = 1.0 / 8.0   # 1/sqrt(DK)

_NC_CACHE = {}


def _build_bass(debug_dumps=False, body_reps=1):
    import concourse.bacc as bacc
    import concourse.tile as tile
    from concourse import mybir

    f32 = mybir.dt.float32
    f32r = mybir.dt.float32r
    Exp = mybir.ActivationFunctionType.Exp

    nc = bacc.Bacc(trn_type="TRN2", debug=False)

    xT = nc.dram_tensor("xT", [D, S], f32, kind="ExternalInput")
    wqT = nc.dram_tensor("wqT", [D, E], f32, kind="ExternalInput")
    wkT = nc.dram_tensor("wkT", [D, E], f32, kind="ExternalInput")
    wvT = nc.dram_tensor("wvT", [D, 256], f32, kind="ExternalInput")
    bq = nc.dram_tensor("bq", [1, E], f32, kind="ExternalInput")
    bk = nc.dram_tensor("bk", [1, E], f32, kind="ExternalInput")
    bv = nc.dram_tensor("bv", [1, 256], f32, kind="ExternalInput")
    woT = nc.dram_tensor("woT", [E, D], f32, kind="ExternalInput")
    ones_d = nc.dram_tensor("ones", [P, 512], f32, kind="ExternalInput")
    outT = nc.dram_tensor("outT", [D, S], f32, kind="ExternalOutput")
    if debug_dumps:
        qt_dump = nc.dram_tensor("qt_dump", [E, S], f32, kind="ExternalOutput")
        kt_dump = nc.dram_tensor("kt_dump", [E, S], f32, kind="ExternalOutput")
        v_dump = nc.dram_tensor("v_dump", [S, HG * 65], f32, kind="ExternalOutput")
        et_dump = nc.dram_tensor("et_dump", [P, 1024], f32, kind="ExternalOutput")
        sc_dump = nc.dram_tensor("sc_dump", [P, 1024], f32, kind="ExternalOutput")
        ctx_dump = nc.dram_tensor("ctx_dump", [E, S], f32, kind="ExternalOutput")
        cps_dump = nc.dram_tensor("cps_dump", [65, 512], f32, kind="ExternalOutput")
        r_dump = nc.dram_tensor("r_dump", [1, 512], f32, kind="ExternalOutput")
        rb_dump = nc.dram_tensor("rb_dump", [64, 512], f32, kind="ExternalOutput")

    xT_d = xT.ap().rearrange("(c p) s -> c p s", p=P)
    wqT_d = wqT.ap().rearrange("(c p) e -> c p e", p=P)
    wkT_d = wkT.ap().rearrange("(c p) e -> c p e", p=P)
    wvT_d = wvT.ap().rearrange("(c p) e -> c p e", p=P)
    outT_d = outT.ap().rearrange("(c p) s -> c p s", p=P)

    with tile.TileContext(nc) as tc:
        for _rep in range(body_reps):
            with tc.tile_pool(name="persist", bufs=1) as persist, \
                 tc.tile_pool(name="work", bufs=4) as work, \
                 tc.tile_pool(name="small", bufs=2) as small, \
                 tc.tile_pool(name="dbg", bufs=2) as dbgp:

                # ---- load inputs (f32r via dtype-punned DMA: PE truncates) ----
                x_sb = []
                for d in range(KD):
                    t = persist.tile([P, S], f32r, tag=f"x{d}")
                    nc.sync.dma_start(out=t[:], in_=xT_d[d].bitcast(f32r))
                    x_sb.append(t)
                wq_sb, wk_sb, wv_sb = [], [], []
                for d in range(KD):
                    t = persist.tile([P, E], f32r, tag=f"wq{d}")
                    nc.sync.dma_start(out=t[:], in_=wqT_d[d].bitcast(f32r))
                    wq_sb.append(t)
                    t = persist.tile([P, E], f32r, tag=f"wk{d}")
                    nc.sync.dma_start(out=t[:], in_=wkT_d[d].bitcast(f32r))
                    wk_sb.append(t)
                    t = persist.tile([P, 256], f32r, tag=f"wv{d}")
                    nc.sync.dma_start(out=t[:], in_=wvT_d[d].bitcast(f32r))
                    wv_sb.append(t)
                bq_sb = persist.tile([1, E], f32r, tag="bq")
                nc.sync.dma_start(out=bq_sb[:], in_=bq.ap().bitcast(f32r))
                bk_sb = persist.tile([1, E], f32r, tag="bk")
                nc.sync.dma_start(out=bk_sb[:], in_=bk.ap().bitcast(f32r))
                bv_sb = persist.tile([1, 256], f32r, tag="bv")
                nc.sync.dma_start(out=bv_sb[:], in_=bv.ap().bitcast(f32r))
                wo_a = persist.tile([P, D], f32r, tag="wo_a")
                nc.sync.dma_start(out=wo_a[:], in_=woT.ap()[0:P, :].bitcast(f32r))
                wo_b = persist.tile([64, D], f32r, tag="wo_b")
                nc.sync.dma_start(out=wo_b[:], in_=woT.ap()[P:E, :].bitcast(f32r))

                ones = persist.tile([P, 512], f32r, tag="ones")
                nc.sync.dma_start(out=ones[:], in_=ones_d.ap().bitcast(f32r))

                # ---- persistent activations ----
                qt_a = persist.tile([P, S], f32r, tag="qt_a")   # heads 0,1
                qt_b = persist.tile([64, S], f32r, tag="qt_b")  # head 2
                kt_a = persist.tile([P, S], f32r, tag="kt_a")
                kt_b = persist.tile([64, S], f32r, tag="kt_b")
                v_sb = [persist.tile([P, HG, 65], f32r, tag=f"v{i}", name=f"v{i}") for i in range(SKC)]
                ctx_a = persist.tile([P, S], f32r, tag="ctx_a")
                ctx_b = persist.tile([64, S], f32r, tag="ctx_b")

                # ================= QKV projections =================
                with tc.tile_pool(name="proj_ps", bufs=8, space="PSUM") as proj_ps:
                    for (w_chunks, b_tile, dst_a, dst_b) in (
                        (wq_sb, bq_sb, qt_a, qt_b),
                        (wk_sb, bk_sb, kt_a, kt_b),
                    ):
                        ps = []
                        for m in range(2):  # e-tiles: [0:128], [128:192]
                            mw = P if m == 0 else 64
                            for c in range(SQC):
                                ps.append(proj_ps.tile([mw, 512], f32, tag="proj", name=f"proj_ps_{m}_{c}"))
                        for d in range(KD):
                            k = 0
                            for m in range(2):
                                mw = P if m == 0 else 64
                                for c in range(SQC):
                                    nc.tensor.matmul(
                                        ps[k][:],
                                        w_chunks[d][:, m * P : m * P + mw],
                                        x_sb[d][:, c * 512 : (c + 1) * 512],
                                        start=(d == 0), stop=False,
                                    )
                                    k += 1
                        k = 0
                        for m in range(2):
                            mw = P if m == 0 else 64
                            for c in range(SQC):
                                nc.tensor.matmul(
                                    ps[k][:],
                                    b_tile[0:1, m * P : m * P + mw],
                                    ones[0:1, 0:512],
                                    start=False, stop=True,
                                )
                                k += 1
                        k = 0
                        for m in range(2):
                            mw = P if m == 0 else 64
                            dst = dst_a if m == 0 else dst_b
                            for c in range(SQC):
                                nc.vector.tensor_copy(
                                    dst[0:mw, c * 512 : (c + 1) * 512], ps[k][:]
                                )
                                k += 1

                    if debug_dumps:
                        nc.sync.dma_start(out=qt_dump.ap()[0:P, :].bitcast(f32r), in_=qt_a[:])
                        nc.sync.dma_start(out=qt_dump.ap()[P:E, :].bitcast(f32r), in_=qt_b[:])
                        nc.sync.dma_start(out=kt_dump.ap()[0:P, :].bitcast(f32r), in_=kt_a[:])
                        nc.sync.dma_start(out=kt_dump.ap()[P:E, :].bitcast(f32r), in_=kt_b[:])


                if debug_dumps:
                    v_dump_d = v_dump.ap().rearrange("(i p) m -> i p m", p=P)
                    for i in range(SKC):
                        nc.sync.dma_start(
                            out=v_dump_d[i].bitcast(f32r),
                            in_=v_sb[i][:].rearrange("p h m -> p (h m)"),
                        )

                # ================= attention =================
                with tc.tile_pool(name="sc_ps", bufs=2, space="PSUM") as sc_ps, \
                     tc.tile_pool(name="ctx_ps", bufs=4, space="PSUM") as ctx_ps:
                    for h in range(HG):
                        if h < 2:
                            kt_h = kt_a[h * 64 : (h + 1) * 64, :]
                            qt_h = qt_a[h * 64 : (h + 1) * 64, :]
                            ctx_h = ctx_a[h * 64 : (h + 1) * 64, :]
                        else:
                            kt_h = kt_b[0:64, :]
                            qt_h = qt_b[0:64, :]
                            ctx_h = ctx_b[0:64, :]

                        cps = [ctx_ps.tile([65, 512], f32, tag="ctx", name=f"cps_{h}_{c}") for c in range(SQC)]
                        for i in range(SKC):
                            sps, ets = [], []
                            for half in range(2):
                                sp = sc_ps.tile([P, 1024], f32, tag="sc", name=f"sp_{h}_{i}_{half}")
                                for j in range(2):
                                    nc.tensor.matmul(
                                        sp[:, j * 512 : (j + 1) * 512],
                                        kt_h[:, i * P : (i + 1) * P],
                                        qt_h[:, half * 1024 + j * 512 : half * 1024 + (j + 1) * 512],
                                        start=True, stop=True,
                                    )
                                sps.append(sp)
                            if h == 0:
                                # V projection for sk-tile i, interleaved into
                                # the ACT-paced attention pipeline (PE slack)
                                vps = sc_ps.tile([P, 256], f32, tag="sc", name=f"vps_{i}")
                                for d in range(KD):
                                    nc.tensor.matmul(
                                        vps[:],
                                        x_sb[d][:, i * P : (i + 1) * P],
                                        wv_sb[d][:],
                                        start=(d == 0), stop=False,
                                    )
                                nc.tensor.matmul(
                                    vps[:], ones[0:1, 0:P], bv_sb[0:1, :],
                                    start=False, stop=True,
                                )
                                nc.vector.tensor_copy(
                                    v_sb[i][:, :, 64:65], ones[:, 0:3][:, :, None]
                                )
                                nc.vector.tensor_copy(
                                    v_sb[i][:, :, 0:64],
                                    vps[:, 0:E].rearrange("p (h d) -> p h d", h=HG),
                                )
                            for half in range(2):
                                et = work.tile([P, 1024], f32r, tag="exp", name=f"et_{h}_{i}_{half}")
                                if debug_dumps and h == 0 and i == 0 and half == 0:
                                    scd = dbgp.tile([P, 1024], f32, tag="scd", name="scd")
                                    nc.vector.tensor_copy(scd[:], sps[half][:])
                                    nc.sync.dma_start(out=sc_dump.ap(), in_=scd[:])
                                nc.scalar.activation(et[:], sps[half][:], Exp, scale=SCALE)
                                if debug_dumps and h == 0 and i == 0 and half == 0:
                                    nc.sync.dma_start(out=et_dump.ap().bitcast(f32r), in_=et[:])
                                ets.append(et)
                            for half in range(2):
                                for j in range(2):
                                    c = half * 2 + j
                                    nc.tensor.matmul(
                                        cps[c][:],
                                        v_sb[i][:, h, :],
                                        ets[half][:, j * 512 : (j + 1) * 512],
                                        start=(i == 0), stop=(i == SKC - 1),
                                    )
                        if debug_dumps and h == 0:
                            cpd = dbgp.tile([65, 512], f32, tag="cpd", name="cpd")
                            nc.vector.tensor_copy(cpd[:], cps[0][:])
                            nc.sync.dma_start(out=cps_dump.ap(), in_=cpd[:])
                        for c in range(SQC):
                            den = small.tile([1, 512], f32, tag="den")
                            nc.vector.tensor_copy(den[:], cps[c][64:65, :])
                            r = small.tile([1, 512], f32, tag="r")
                            nc.vector.reciprocal_approx_fast(r[:], den[:])
                            rb = small.tile([64, 512], f32, tag="rb")
                            nc.gpsimd.partition_broadcast(rb[:], r[:])
                            nc.vector.tensor_mul(
                                ctx_h[:, c * 512 : (c + 1) * 512],
                                cps[c][0:64, :],
                                rb[:],
                            )
                            if debug_dumps and h == 0 and c == 0:
                                nc.sync.dma_start(out=r_dump.ap(), in_=r[:])
                                nc.sync.dma_start(out=rb_dump.ap(), in_=rb[:])
                            if h == HG - 1:
                                # output projection for this sq chunk (all heads done)
                                for e in range(KD):
                                    op = ctx_ps.tile([P, 512], f32, tag="ctx",
                                                     name=f"op_{e}_{c}")
                                    nc.tensor.matmul(
                                        op[:],
                                        wo_a[:, e * P : (e + 1) * P],
                                        ctx_a[:, c * 512 : (c + 1) * 512],
                                        start=True, stop=False,
                                    )
                                    nc.tensor.matmul(
                                        op[:],
                                        wo_b[:, e * P : (e + 1) * P],
                                        ctx_b[:, c * 512 : (c + 1) * 512],
                                        start=False, stop=True,
                                    )
                                    o = work.tile([P, 512], f32, tag="o", bufs=6)
                                    if e % 2 == 0:
                                        nc.vector.tensor_copy(o[:], op[:])
                                    else:
                                        nc.scalar.activation(
                                            o[:], op[:],
                                            mybir.ActivationFunctionType.Copy,
                                        )
                                    nc.sync.dma_start(
                                        out=outT_d[e][:, c * 512 : (c + 1) * 512], in_=o[:]
                                    )

                if debug_dumps:
                    nc.sync.dma_start(out=ctx_dump.ap()[0:P, :].bitcast(f32r), in_=ctx_a[:])
                    nc.sync.dma_start(out=ctx_dump.ap()[P:E, :].bitcast(f32r), in_=ctx_b[:])


    nc.finalize()
    return nc


def _get_nc(debug_dumps=False, body_reps=1):
    key = ("dbg" if debug_dumps else "nc", body_reps)
    if key not in _NC_CACHE:
        _NC_CACHE[key] = _build_bass(debug_dumps, body_reps)
    return _NC_CACHE[key]


def _core_inputs(c, x, w_q, b_q, w_k, b_k, w_v, b_v, w_o):
    b, g = divmod(c, 4)
    gs = slice(g * E, (g + 1) * E)
    wv_pad = np.zeros((D, 256), np.float32)
    wv_pad[:, :E] = np.ascontiguousarray(w_v[gs, :].T)
    bv_pad = np.zeros((1, 256), np.float32)
    bv_pad[0, :E] = b_v[gs]
    return {
        "xT": np.ascontiguousarray(x[b].T),
        "wqT": np.ascontiguousarray(w_q[gs, :].T),
        "wkT": np.ascontiguousarray(w_k[gs, :].T),
        "wvT": wv_pad,
        "bq": b_q[gs].reshape(1, E).astype(np.float32),
        "bk": b_k[gs].reshape(1, E).astype(np.float32),
        "bv": bv_pad,
        "woT": np.ascontiguousarray(w_o[:, gs].T),
        "ones": np.ones((P, 512), np.float32),
    }


def kernel(x, w_q, b_q, w_k, b_k, w_v, b_v, w_o, b_o, _trace=False, _debug=False):
    from concourse.bass_utils import run_bass_kernel_spmd

    x = np.asarray(x, np.float32)
    args = [np.asarray(a, np.float32) for a in
            (w_q, b_q, w_k, b_k, w_v, b_v, w_o)]
    b_o = np.asarray(b_o, np.float32)

    nc = _get_nc(_debug)
    in_maps = [_core_inputs(c, x, *args) for c in range(8)]
    res = run_bass_kernel_spmd(nc, in_maps, core_ids=list(range(8)), trace=_trace)

    out = np.zeros((B, S, D), np.float32)
    for c in range(8):
        out[c // 4] += res.results[c]["outT"].T
    out += b_o
    if _trace:
        kernel._last_results = res
    return out

